# revision 70
# baseline (speedup 1.0000x reference)
"""Trainium2 Bass kernel for nn_Attention_local — v3.

Data-parallel over batch: 8 images -> 8 NeuronCores, no collectives.

Changes vs v2 (510us): the v path no longer folds the depthwise conv into
PE matmuls (which cost 6x the minimal 1x1 work in bf16). Instead v = bf16
1x1 (strip-wise, 2 K-chunks) followed by all 9 depthwise taps as PACKED
raw-layout DVE ops (4x perf mode), then a strided phase-scatter pass and a
DMA to DRAM staging. The v strips interleave with the fp8 q,k fold so the
vector engines chew taps while PE does fold matmuls. AV-output staging to
o1/o2 is now direct SBUF->SBUF DMA (no DRAM round trip).

Layouts (unchanged from v2):
- d-order per head: d = c_local*16 + (fy*4+fx); n = h1*32+w1.
- x uploaded zero-padded to 130x130 (origin (1,1)); ones-channel carries
  biases with exact zero-padding semantics.
"""

import numpy as np

HEADS = 4
C = 192
HW = 128
NPIX = HW * HW
PW = 130
PAD = PW * PW            # 16900
G = 32
NP = G * G               # 1024
PH = 16
DH = 48 * PH             # 768
S = 128.0                # fp8 weight scale for the qk fold (cancels in norms)
EPS = 1e-12

SH = 16                  # v-strip height (image rows)
NS = HW // SH            # 8 strips
SROWS = SH + 2           # yp rows per strip (halo)
SF = SROWS * PW          # 2340
YF = SF + 2              # yp tile: spare pad col at flat 0 + tail slack
VOF = PH * PW            # 2080 (raw tap output incl pad cols)
VF = PH * HW             # 2048 (phase-major vout strip free size)

TAPS = [(dy, dx) for dy in (-1, 0, 1) for dx in (-1, 0, 1)]

_COMPILED = {}


def _build(dbg=False):
    import concourse.bass as bass
    import concourse.bacc as bacc
    import concourse.mybir as mybir
    from concourse.tile import TileContext
    from concourse.masks import make_identity
    from contextlib import ExitStack

    F32 = mybir.dt.float32
    BF16 = mybir.dt.bfloat16
    FP8 = mybir.dt.float8e4
    AF = mybir.ActivationFunctionType
    ALU = mybir.AluOpType
    DR = mybir.MatmulPerfMode.DoubleRow
    AP = bass.AP

    nc = bacc.Bacc("TRN2", target_bir_lowering=False, debug=False)

    xpk_d = nc.dram_tensor("xpk", [128, 2 * PAD], FP8, kind="ExternalInput")
    xb1_d = nc.dram_tensor("xb1", [128, PAD], BF16, kind="ExternalInput")
    xb2_d = nc.dram_tensor("xb2", [65, PAD], BF16, kind="ExternalInput")
    wq8_d = nc.dram_tensor("wq8", [128, 2 * 9 * 384], FP8, kind="ExternalInput")
    wva_d = nc.dram_tensor("wva", [128, C], BF16, kind="ExternalInput")
    wvb_d = nc.dram_tensor("wvb", [65, C], BF16, kind="ExternalInput")
    wvfa_d = nc.dram_tensor("wvfa", [128, 2 * C], BF16, kind="ExternalInput")
    wvfb_d = nc.dram_tensor("wvfb", [65, 2 * C], BF16, kind="ExternalInput")
    dwv_d = nc.dram_tensor("dwv", [C, 10], F32, kind="ExternalInput")
    tpc_d = nc.dram_tensor("tpc", [C, 1], F32, kind="ExternalInput")
    pta_d = nc.dram_tensor("pta", [128, C], BF16, kind="ExternalInput")
    ptb_d = nc.dram_tensor("ptb", [65, C], BF16, kind="ExternalInput")
    ones8_d = nc.dram_tensor("ones8", [128, 8], BF16, kind="ExternalInput")
    ones16k_d = nc.dram_tensor("ones16k", [1, NPIX], BF16, kind="ExternalInput")
    y_d = nc.dram_tensor("y", [C, NPIX], BF16, kind="ExternalOutput")
    if dbg:
        natd_d = nc.dram_tensor("natd", [384, NPIX], BF16, kind="ExternalOutput")
        vstd_d = nc.dram_tensor("vstd", [HEADS * DH, NP], BF16,
                                kind="ExternalOutput")
        o1d_d = nc.dram_tensor("o1d", [128, NPIX], BF16, kind="ExternalOutput")
        o2d_d = nc.dram_tensor("o2d", [65, NPIX], BF16, kind="ExternalOutput")

    # row-run maps for the qk transpose copies (chunk m covers qk rows
    # m*128..m*128+128; rows 0..191 q, 192..383 k).
    def chunk_runs(m):
        runs = []
        r = m * 128
        end = r + 128
        while r < end:
            tens = 0 if r < 192 else 1
            c = r if tens == 0 else r - 192
            h, cl = c // 48, c % 48
            take = min(48 - cl, (192 if tens == 0 else 384) - r, end - r)
            runs.append((r - m * 128, take, tens, h * DH + cl * 16))
            r += take
        return runs

    def norm_parts(m):
        parts = []
        r = m * 128
        end = r + 128
        while r < end:
            if r < 192:
                take = min(192 - r, end - r)
                parts.append((r - m * 128, take, True, r))
            else:
                take = end - r
                parts.append((r - m * 128, take, False, r - 192))
            r += take
        return parts

    with TileContext(nc) as tc:
        with ExitStack() as es_all:
            pers = es_all.enter_context(tc.tile_pool(name="pers", bufs=1))
            qt = [pers.tile([128, 2, 4 * DH], FP8, name=f"qt{g}", tag=f"qt{g}")
                  for g in range(4)]
            kt = [pers.tile([128, 2, 4 * DH], FP8, name=f"kt{g}", tag=f"kt{g}")
                  for g in range(4)]
            kne = pers.tile([128, 24], F32, tag="kne")
            ident = pers.tile([128, 128], BF16, tag="ident")
            make_identity(nc, ident)
            ones8 = pers.tile([128, 8], BF16, tag="ones8")
            nc.sync.dma_start(ones8[:], ones8_d.ap())


            dram = es_all.enter_context(
                tc.tile_pool(name="dram", bufs=1, space="DRAM"))
            vst_d = dram.tile([HEADS * DH, NP], BF16, tag="vst")
            ost_d = dram.tile([HEADS * DH, NP], BF16, tag="ost")

            # ============== phase AB: fp8 qk fold + bf16 v strips =========
            with ExitStack() as esAB:
                wp = esAB.enter_context(tc.tile_pool(name="wp", bufs=1))
                wq8 = wp.tile([128, 2, 9, 384], FP8, tag="wq8")
                nc.sync.dma_start(
                    wq8[:].rearrange("p a t m -> p (a t m)"), wq8_d.ap())
                xq = wp.tile([128, 2, PAD], FP8, tag="xq")
                for a, b in ((0, 4290), (4290, 8580), (8580, 12870),
                             (12870, PAD)):
                    nc.sync.dma_start(
                        AP(xq.tensor, a, [[2 * PAD, 128], [PAD, 2], [1, b - a]]),
                        AP(xpk_d.ap().tensor, a,
                           [[2 * PAD, 128], [PAD, 2], [1, b - a]]))
                tpcs = wp.tile([128, 2], F32, tag="tpcs")
                nc.sync.dma_start(tpcs[0:128, 0:1], tpc_d.ap()[0:128, :])
                nc.sync.dma_start(tpcs[0:64, 1:2], tpc_d.ap()[128:192, :])

                wva = wp.tile([128, C], BF16, tag="wva")
                wvb = wp.tile([65, C], BF16, tag="wvb")
                nc.sync.dma_start(wva[:], wva_d.ap())
                nc.sync.dma_start(wvb[:], wvb_d.ap())
                wvfa = wp.tile([128, 2, C], BF16, tag="wvfa")
                wvfb = wp.tile([65, 2, C], BF16, tag="wvfb")
                nc.sync.dma_start(
                    wvfa[:].rearrange("p a c -> p (a c)"), wvfa_d.ap())
                nc.sync.dma_start(
                    wvfb[:].rearrange("p a c -> p (a c)"), wvfb_d.ap())
                dvA = wp.tile([128, 10], F32, tag="dvA")
                dvB = wp.tile([128, 10], F32, tag="dvB")
                nc.sync.dma_start(dvA[:], dwv_d.ap()[0:128, :])
                nc.sync.dma_start(dvB[0:64, :], dwv_d.ap()[128:192, :])
                nc.sync.dma_start(dvB[64:128, :], dwv_d.ap()[128:192, :])

                natp = esAB.enter_context(tc.tile_pool(name="natp", bufs=1))
                sqp = esAB.enter_context(tc.tile_pool(name="sqp", bufs=1))
                nrmp = esAB.enter_context(tc.tile_pool(name="nrmp", bufs=2))
                psA = esAB.enter_context(
                    tc.tile_pool(name="psA", bufs=3, space="PSUM"))
                psT = esAB.enter_context(
                    tc.tile_pool(name="psT", bufs=2, space="PSUM"))

                xsp = esAB.enter_context(tc.tile_pool(name="xsp", bufs=1))
                tmpp = esAB.enter_context(tc.tile_pool(name="tmpp", bufs=5))
                ypp = esAB.enter_context(tc.tile_pool(name="ypp", bufs=2))
                votp = esAB.enter_context(tc.tile_pool(name="votp", bufs=1))
                psV = esAB.enter_context(
                    tc.tile_pool(name="psV", bufs=2, space="PSUM"))
                psF = esAB.enter_context(
                    tc.tile_pool(name="psF", bufs=1, space="PSUM"))

                XF = 2 * PAD
                WF = 2 * 9 * 384
                QF = 2 * 4 * DH

                # ---- v strip machinery (per-strip, base-0 matmuls) ----
                def v_1x1(ps):
                    out = []
                    for si in range(2):
                        s_ = 2 * ps + si
                        c0 = SH * s_ * PW
                        xsa = xsp.tile([128, SF], BF16, name=f"xsa{si}",
                                       tag=f"xsa{si}")
                        xsb = xsp.tile([65, SF], BF16, name=f"xsb{si}",
                                       tag=f"xsb{si}")
                        nc.sync.dma_start(xsa[:], xb1_d.ap()[:, c0:c0 + SF])
                        nc.sync.dma_start(xsb[:], xb2_d.ap()[:, c0:c0 + SF])
                        ypA = ypp.tile([128, YF], BF16, name=f"ypA{si}",
                                       tag=f"ypA{si}")
                        ypB = ypp.tile([64, YF], BF16, name=f"ypB{si}",
                                       tag=f"ypB{si}", bufs=1)
                        for yp, cnt in ((ypA, 128), (ypB, 64)):
                            nc.gpsimd.memset(
                                AP(yp.tensor, 1, [[YF, cnt], [PW, SROWS]]), 0.0)
                            nc.gpsimd.memset(
                                AP(yp.tensor, PW, [[YF, cnt], [PW, SROWS]]),
                                0.0)
                            nc.gpsimd.memset(
                                AP(yp.tensor, 0, [[YF, cnt], [1, 1]]), 0.0)
                        vos = {}
                        for yp, cnt, m0, cb in ((ypA, 128, 0, 0),
                                                (ypB, 64, 128, 1)):
                            for t in range(4):
                                pv = psV.tile([128, 512], F32, tag="psV")
                                rhs1 = AP(xsa.tensor, (4 * t + 1) * PW + 1,
                                          [[SF, 128], [PW, 4], [1, 128]])
                                nc.tensor.matmul(pv[0:cnt, :],
                                                 wva[:, m0:m0 + cnt], rhs1,
                                                 start=True, stop=False)
                                rhs2 = AP(xsb.tensor, (4 * t + 1) * PW + 1,
                                          [[SF, 65], [PW, 4], [1, 128]])
                                nc.tensor.matmul(pv[0:cnt, :],
                                                 wvb[:, m0:m0 + cnt], rhs2,
                                                 start=False, stop=True)
                                dst = AP(yp.tensor, 1 + (4 * t + 1) * PW + 1,
                                         [[YF, cnt], [PW, 4], [1, 128]])
                                nc.scalar.copy(dst, pv[0:cnt, :])
                            ph = psV.tile([128, 512], F32, tag="psV")
                            rh1 = AP(xsa.tensor, 1,
                                     [[SF, 128], [17 * PW, 2], [1, 128]])
                            nc.tensor.matmul(ph[0:cnt, 0:256],
                                             wva[:, m0:m0 + cnt], rh1,
                                             start=True, stop=False)
                            rh2 = AP(xsb.tensor, 1,
                                     [[SF, 65], [17 * PW, 2], [1, 128]])
                            nc.tensor.matmul(ph[0:cnt, 0:256],
                                             wvb[:, m0:m0 + cnt], rh2,
                                             start=False, stop=True)
                            nc.scalar.copy(
                                AP(yp.tensor, 1 + 1,
                                   [[YF, cnt], [17 * PW, 2], [1, 128]]),
                                ph[0:cnt, 0:256])
                            # folded taps (1,3) -> vo init via PE
                            key = f"{s_ % 2}{cb}"
                            HVO = 8 * PW
                            for hf in range(2):
                                vo = votp.tile([cnt, HVO], BF16,
                                               name=f"vo{key}{hf}",
                                               tag=f"vo{key}{hf}", bufs=1)
                                vos[(cb, hf)] = vo
                                nc.gpsimd.memset(
                                    AP(vo.tensor, 0, [[HVO, cnt], [PW, 8]]),
                                    0.0)
                                nc.gpsimd.memset(
                                    AP(vo.tensor, PW - 1,
                                       [[HVO, cnt], [PW, 8]]), 0.0)
                                for th in range(2):
                                    T = hf * 2 + th
                                    pf = psF.tile([128, 512], F32, tag="psF")
                                    for i, (dy, dx) in enumerate(
                                            (TAPS[1], TAPS[3])):
                                        off = (4 * T + 1 + dy) * PW + 1 + dx
                                        rf1 = AP(xsa.tensor, off,
                                                 [[SF, 128], [PW, 4],
                                                  [1, 128]])
                                        nc.tensor.matmul(
                                            pf[0:cnt, :],
                                            wvfa[:, i, m0:m0 + cnt], rf1,
                                            start=(i == 0), stop=False)
                                        rf2 = AP(xsb.tensor, off,
                                                 [[SF, 65], [PW, 4],
                                                  [1, 128]])
                                        nc.tensor.matmul(
                                            pf[0:cnt, :],
                                            wvfb[:, i, m0:m0 + cnt], rf2,
                                            start=False, stop=(i == 1))
                                    dstf = AP(vo.tensor, th * 4 * PW + 1,
                                              [[HVO, cnt], [PW, 4], [1, 128]])
                                    nc.scalar.copy(dstf, pf[0:cnt, :])
                        out.append((ypA, ypB, s_, vos))
                    return out

                def v_taps_gen(ps, pairs):
                    HVO = 8 * PW
                    for ypA, ypB, s_, vos in pairs:
                        for yp, dv, cnt, cb in ((ypA, dvA, 128, 0),
                                                (ypB, dvB, 64, 1)):
                            key = f"{s_ % 2}{cb}"
                            for hf in range(2):
                                vo = vos[(cb, hf)]
                                dst = AP(vo.tensor, 0, [[HVO, cnt], [1, HVO]])

                                def srcof(dy, dx, yp=yp, hf=hf, cnt=cnt):
                                    return AP(yp.tensor,
                                              1 + (1 + dy + 8 * hf) * PW + dx,
                                              [[YF, cnt], [1, HVO]])

                                ptaps = (0, 5, 7)
                                ptmp = {}
                                for tap in ptaps:
                                    dy, dx = TAPS[tap]
                                    tm = tmpp.tile([128, HVO], BF16,
                                                   name="tm", tag="tm")
                                    tma = AP(tm.tensor, 0,
                                             [[HVO, cnt], [1, HVO]])
                                    ptmp[tap] = tma
                                    def op(tma=tma, src=srcof(dy, dx),
                                           w=dv[0:cnt, tap:tap + 1]):
                                        nc.gpsimd.tensor_scalar(
                                            tma, src, w, None, op0=ALU.mult)
                                    yield op
                                tmc = tmpp.tile([128, HVO], BF16,
                                                name="tm", tag="tm")
                                tmca = AP(tmc.tensor, 0,
                                          [[HVO, cnt], [1, HVO]])
                                def op(tmca=tmca, src=srcof(0, 0),
                                       w=dv[0:cnt, 4:5],
                                       b=dv[0:cnt, 9:10]):
                                    nc.vector.tensor_scalar(
                                        tmca, src, w, b,
                                        op0=ALU.mult, op1=ALU.add)
                                yield op
                                def op2(dst=dst, tmca=tmca):
                                    nc.vector.tensor_tensor(
                                        dst, dst, tmca, op=ALU.add)
                                yield op2
                                for tap in ptaps:
                                    def op2(dst=dst, tma=ptmp[tap]):
                                        nc.vector.tensor_tensor(
                                            dst, dst, tma, op=ALU.add)
                                    yield op2
                                for tap in (2, 6, 8):
                                    dy, dx = TAPS[tap]
                                    tm = tmpp.tile([128, HVO], BF16,
                                                   name="tm", tag="tm")
                                    tma = AP(tm.tensor, 0,
                                             [[HVO, cnt], [1, HVO]])
                                    def op(tma=tma, src=srcof(dy, dx),
                                           w=dv[0:cnt, tap:tap + 1]):
                                        nc.vector.tensor_scalar(
                                            tma, src, w, None, op0=ALU.mult)
                                    yield op
                                    def op2(dst=dst, tma=tma):
                                        nc.vector.tensor_tensor(
                                            dst, dst, tma, op=ALU.add)
                                    yield op2
                                vp = votp.tile([cnt, 1024], BF16,
                                               name=f"vp{key}",
                                               tag=f"vp{key}", bufs=1)
                                for fy in range(4):
                                    srcs = AP(vo.tensor, fy * PW + 1,
                                              [[HVO, cnt], [1, 4],
                                               [4 * PW, 2], [4, 32]])
                                    dsts = AP(vp.tensor, fy * 256,
                                              [[1024, cnt], [64, 4],
                                               [32, 2], [1, 32]])
                                    if fy == 0:
                                        def op(dsts=dsts, srcs=srcs):
                                            with tc.high_priority(offset=50000):
                                                nc.scalar.copy(dsts, srcs)
                                    else:
                                        def op(dsts=dsts, srcs=srcs):
                                            with tc.high_priority(offset=50000):
                                                nc.gpsimd.tensor_copy(dsts,
                                                                      srcs)
                                    yield op

                                def dma(vp=vp, s_=s_, hf=hf, cnt=cnt, cb=cb):
                                    vt = vst_d[:].tensor
                                    dstd = AP(vt,
                                              cb * 2048 * NP + s_ * 128
                                              + hf * 64,
                                              [[16 * NP, cnt], [NP, 16],
                                               [1, 64]])
                                    srcd = AP(vp.tensor, 0,
                                              [[1024, cnt], [64, 16], [1, 64]])
                                    with tc.high_priority(offset=50000):
                                        nc.sync.dma_start(dstd, srcd)
                                yield dma

                pending_v = []

                def pump(n):
                    for _ in range(n):
                        if not pending_v:
                            return
                        try:
                            op = next(pending_v[0])
                            op()
                        except StopIteration:
                            pending_v.pop(0)

                def start_pair(ps):
                    pairs = v_1x1(ps)
                    pending_v.append(v_taps_gen(ps, pairs))

                # ---- qk fold ----
                start_pair(0)
                start_pair(1)
                for m in range(3):
                    nat = natp.tile([128, NPIX], BF16, tag="nat")
                    for t in range(32):
                        ps_ = psA.tile([128, 512], F32, tag="psA")
                        for tap, (dy, dx) in enumerate(TAPS):
                            rhs = AP(xq.tensor,
                                     (4 * t + 1 + dy) * PW + 1 + dx,
                                     [[XF, 128], [PAD, 2], [PW, 4], [1, 128]])
                            lhsT = AP(wq8.tensor, tap * 384 + m * 128,
                                      [[WF, 128], [9 * 384, 2], [1, 128]])
                            nc.tensor.matmul(ps_[:], lhsT, rhs,
                                             start=(tap == 0), stop=(tap == 8),
                                             perf_mode=DR)
                        src = AP(ps_.tensor, 0,
                                 [[512, 128], [128, 4], [4, 32], [1, 4]])
                        dst = AP(nat.tensor, t * G,
                                 [[NPIX, 128], [4 * NP, 4], [1, 32], [NP, 4]])
                        with tc.high_priority(offset=90000):
                            if t % 4 != 3:
                                nc.scalar.copy(dst, src)
                            else:
                                nc.vector.tensor_copy(dst, src)
                        pump(5)

                    # interleave next v strip pair's PE work before norms
                    if m < 2:
                        start_pair(m + 2)

                    # ---- norms ----
                    nrm = nrmp.tile([128, 16], F32, tag="nrm")
                    es_hp = tc.high_priority(offset=100000)
                    es_hp.__enter__()
                    for p in range(PH):
                        sq = sqp.tile([128, NP], BF16, tag="sq")
                        nc.scalar.activation(sq[:], nat[:, p * NP:(p + 1) * NP],
                                             AF.Square,
                                             accum_out=nrm[:, p:p + 1])
                        nc.scalar.sqrt(nrm[:, p:p + 1], nrm[:, p:p + 1])
                        nc.vector.tensor_scalar_max(
                            nrm[:, p:p + 1], nrm[:, p:p + 1], EPS)
                        nc.vector.reciprocal(nrm[:, p:p + 1], nrm[:, p:p + 1])
                        for (r0, cnt, is_q, ch0) in norm_parts(m):
                            if is_q:
                                tsl = tpcs[r0:r0 + cnt, 0:1] if m == 0 \
                                    else tpcs[0:cnt, 1:2]
                                nc.vector.tensor_scalar_mul(
                                    nrm[r0:r0 + cnt, p:p + 1],
                                    nrm[r0:r0 + cnt, p:p + 1], tsl)
                                nc.vector.tensor_scalar_mul(
                                    nat[r0:r0 + cnt, p * NP:(p + 1) * NP],
                                    nat[r0:r0 + cnt, p * NP:(p + 1) * NP],
                                    nrm[r0:r0 + cnt, p:p + 1])
                        pump(1)
                    es_hp.__exit__(None, None, None)
                    for (r0, cnt, is_q, ch0) in norm_parts(m):
                        if not is_q:
                            for gi in range(cnt // 8):
                                h = (ch0 + gi * 8) // 48
                                clh = (ch0 + gi * 8) % 48
                                col = h * 6 + (clh * 16) // 128
                                src = AP(nrm.tensor, (r0 + gi * 8) * 16,
                                         [[16, 8], [1, 16]])
                                with tc.high_priority(offset=70000):
                                    nc.sync.dma_start(
                                        kne[0:128, col:col + 1], src)

                    # ---- transposes ----
                    runs = chunk_runs(m)
                    for pg in range(2):
                        for nb in range(8):
                            tp = psT.tile([128, 1024], BF16, tag="psT")
                            for pq in range(8):
                                p = pg * 8 + pq
                                nc.tensor.transpose(
                                    tp[:, pq * 128:(pq + 1) * 128],
                                    nat[:, p * NP + nb * 128:
                                        p * NP + (nb + 1) * 128],
                                    ident[:])
                            for ri, (c0, ccnt, tens, dstbase) in enumerate(runs):
                                tgt = (qt if tens == 0 else kt)[nb // 2]
                                srcc = AP(tp.tensor, c0,
                                          [[1024, 128], [128, 8], [1, ccnt]])
                                dstc = AP(tgt.tensor,
                                          (nb % 2) * 4 * DH + dstbase + pg * 8,
                                          [[QF, 128], [1, 8], [16, ccnt]])
                                with tc.high_priority(offset=80000):
                                    if (nb + ri) % 2 == 0:
                                        nc.scalar.copy(dstc, srcc)
                                    else:
                                        nc.vector.tensor_copy(dstc, srcc)
                            pump(2)
                    if dbg:
                        nc.sync.dma_start(
                            natd_d.ap()[m * 128:(m + 1) * 128, :], nat[:])

                pump(1000)  # drain any remaining v work

            # ============== QK^T + exp, then AV + staging ==============
            with ExitStack() as esC:
                orp = esC.enter_context(tc.tile_pool(name="orp", bufs=1))
                o1 = orp.tile([128, NPIX], BF16, tag="o1")
                o2 = orp.tile([65, NPIX], BF16, tag="o2")
                nc.sync.dma_start(o2[64:65, :], ones16k_d.ap())

                with ExitStack() as esC1:
                    etp = esC1.enter_context(tc.tile_pool(name="etp", bufs=24))
                    psQK = esC1.enter_context(
                        tc.tile_pool(name="psQK", bufs=2, space="PSUM"))
                    vrp = esC1.enter_context(tc.tile_pool(name="vrp", bufs=1))
                    vr = [vrp.tile([128, NP], BF16, name=f"vr{j}", tag=f"vr{j}")
                          for j in range(24)]
                    for j in range(24):
                        nc.sync.dma_start(
                            vr[j][:], vst_d[j * 128:(j + 1) * 128, :])
                    odp = esC1.enter_context(tc.tile_pool(name="odp", bufs=6))
                    zp = esC1.enter_context(tc.tile_pool(name="zp", bufs=4))
                    psAV = esC1.enter_context(
                        tc.tile_pool(name="psAV", bufs=2, space="PSUM"))

                    for h in range(HEADS):
                        ets = []
                        for ec in range(6):
                            for dh in range(2):
                                pa = psQK.tile([128, 384], F32, tag="psQK")
                                for g in range(4):
                                    lhsT = kt[g][:, :, h * DH + ec * 128:
                                                 h * DH + (ec + 1) * 128]
                                    nc.tensor.matmul(
                                        pa[:, 0:384], lhsT,
                                        qt[g][:, :, h * DH + dh * 384:
                                              h * DH + dh * 384 + 384],
                                        start=(g == 0), stop=(g == 3),
                                        perf_mode=DR)
                                et = etp.tile([128, 384], BF16, tag="et",
                                              name="et")
                                nc.scalar.activation(
                                    et[:], pa[:], AF.Exp,
                                    scale=kne[:, h * 6 + ec:h * 6 + ec + 1])
                                ets.append(et)

                        for dc in range(6):
                            po = psAV.tile([128, 1536], F32, tag="psAV")
                            for ec in range(6):
                                st, sp = ec == 0, ec == 5
                                lhsT = ets[ec * 2 + (1 if dc >= 3 else 0)][
                                    :, (dc % 3) * 128:(dc % 3) * 128 + 128]
                                v = vr[h * 6 + ec]
                                nc.tensor.matmul(po[:, 0:512], lhsT,
                                                 v[:, 0:512],
                                                 start=st, stop=sp)
                                nc.tensor.matmul(po[:, 512:1024], lhsT,
                                                 v[:, 512:1024],
                                                 start=st, stop=sp)
                                nc.tensor.matmul(po[:, 1024:1032], lhsT,
                                                 ones8[:], start=st, stop=sp)
                            zr = zp.tile([128, 1], F32, tag="zr")
                            nc.vector.tensor_scalar_add(zr[:], po[:, 1024:1025],
                                                        1.0)
                            nc.vector.reciprocal(zr[:], zr[:])
                            ot = odp.tile([128, NP], BF16, tag="ot")
                            nc.scalar.mul(ot[:], po[:, 0:1024], zr[:])
                            nc.sync.dma_start(
                                ost_d[h * DH + dc * 128:
                                      h * DH + (dc + 1) * 128, :], ot[:])
                            # gather these 8 channel rows into o1/o2 now
                            ostv = ost_d[:].rearrange("(c q) n -> c (q n)",
                                                      q=PH)
                            c0 = h * 48 + dc * 8
                            tgt, b0 = (o1, c0) if c0 < 128 else (o2, c0 - 128)
                            nc.sync.dma_start(tgt[b0:b0 + 8, :],
                                              ostv[c0:c0 + 8, :])

                if dbg:
                    for j in range(24):
                        nc.sync.dma_start(
                            vstd_d.ap()[j * 128:(j + 1) * 128, :],
                            vst_d[j * 128:(j + 1) * 128, :])
                    nc.sync.dma_start(o1d_d.ap(), o1[:])
                    nc.sync.dma_start(o2d_d.ap(), o2[:])

                # ================= proj =================
                with ExitStack() as esC2:
                    pwp = esC2.enter_context(tc.tile_pool(name="pwp", bufs=1))
                    pta = pwp.tile([128, C], BF16, tag="pta")
                    ptb = pwp.tile([65, C], BF16, tag="ptb")
                    nc.sync.dma_start(pta[:], pta_d.ap())
                    nc.sync.dma_start(ptb[:], ptb_d.ap())
                    y1 = pwp.tile([128, NPIX], BF16, tag="y1")
                    y2 = pwp.tile([64, NPIX], BF16, tag="y2")
                    psY = esC2.enter_context(
                        tc.tile_pool(name="psY", bufs=8, space="PSUM"))

                    for t in range(32):
                        for mo, cnt, yt in ((0, 128, y1), (1, 64, y2)):
                            ps_ = psY.tile([128, 512], F32, tag="psY")
                            rhs1 = AP(o1.tensor, t * G,
                                      [[NPIX, 128], [4 * NP, 4], [1, 32],
                                       [NP, 4]])
                            nc.tensor.matmul(ps_[0:cnt, :],
                                             pta[:, mo * 128:mo * 128 + cnt],
                                             rhs1, start=True, stop=False)
                            rhs2 = AP(o2.tensor, t * G,
                                      [[NPIX, 65], [4 * NP, 4], [1, 32],
                                       [NP, 4]])
                            nc.tensor.matmul(ps_[0:cnt, :],
                                             ptb[:, mo * 128:mo * 128 + cnt],
                                             rhs2, start=False, stop=True)
                            if t % 2 == 0:
                                nc.scalar.copy(yt[0:cnt, t * 512:(t + 1) * 512],
                                               ps_[0:cnt, :])
                            else:
                                nc.vector.tensor_copy(
                                    yt[0:cnt, t * 512:(t + 1) * 512],
                                    ps_[0:cnt, :])
                    for q0 in range(0, 32, 2):
                        cs = slice(q0 * 512, (q0 + 2) * 512)
                        nc.sync.dma_start(y_d.ap()[0:128, cs], y1[:, cs])
                        nc.sync.dma_start(y_d.ap()[128:192, cs], y2[:, cs])

    nc.compile()
    return nc


def _prep_common(qkv_w, qkv_b, dw_w, dw_b, proj_w, proj_b, temp):
    import ml_dtypes
    FP8 = ml_dtypes.float8_e4m3
    BF16 = ml_dtypes.bfloat16

    dw9 = dw_w.reshape(576, 9)
    wq8 = np.zeros((128, 2, 9, 384), np.float32)
    for j in range(2):
        for t in range(9):
            w = dw9[:384, t:t + 1] * qkv_w[:384, :]     # [384, 192]
            nch = 128 if j == 0 else 64
            wq8[:nch, j, t, :] = w[:, j * 128:j * 128 + nch].T
        wq8[64, 1, :, :] = (dw9[:384, :] * qkv_b[:384, None]).T
    wq8[64, 1, 4, :] += dw_b[:384]
    wq8 = (S * wq8).astype(FP8)

    wva = np.ascontiguousarray(qkv_w[384:, 0:128].T).astype(BF16)
    wvb = np.zeros((65, C), np.float32)
    wvb[0:64] = qkv_w[384:, 128:192].T
    wvb[64] = qkv_b[384:]
    wvb = wvb.astype(BF16)

    FT = (1, 3)
    wvfa = np.zeros((128, 2, C), np.float32)
    wvfb = np.zeros((65, 2, C), np.float32)
    for i, t in enumerate(FT):
        w = dw9[384:, t:t + 1] * qkv_w[384:, :]
        wvfa[:, i, :] = w[:, 0:128].T
        wvfb[0:64, i, :] = w[:, 128:192].T
        wvfb[64, i, :] = dw9[384:, t] * qkv_b[384:]
    wvfa = wvfa.astype(BF16)
    wvfb = wvfb.astype(BF16)

    dwv = np.zeros((C, 10), np.float32)
    dwv[:, 0:9] = dw9[384:]
    dwv[:, 9] = dw_b[384:]

    pta = np.ascontiguousarray(proj_w[:, 0:128].T).astype(BF16)
    ptb = np.zeros((65, C), np.float32)
    ptb[0:64] = proj_w[:, 128:192].T
    ptb[64] = proj_b
    ptb = ptb.astype(BF16)

    tpc = np.repeat(temp, 48).reshape(C, 1).astype(np.float32)

    return {
        "wq8": np.ascontiguousarray(wq8.reshape(128, 2 * 9 * 384)),
        "wva": wva,
        "wvb": wvb,
        "wvfa": np.ascontiguousarray(wvfa.reshape(128, 2 * C)),
        "wvfb": np.ascontiguousarray(wvfb.reshape(65, 2 * C)),
        "dwv": dwv,
        "tpc": tpc,
        "pta": pta,
        "ptb": ptb,
        "ones8": np.ones((128, 8), BF16),
        "ones16k": np.ones((1, NPIX), BF16),
    }


def _prep_x(xb):
    import ml_dtypes
    FP8 = ml_dtypes.float8_e4m3
    BF16 = ml_dtypes.bfloat16
    xp = np.zeros((C, PW, PW), np.float32)
    xp[:, 1:129, 1:129] = xb
    ones = np.zeros((PW, PW), np.float32)
    ones[1:129, 1:129] = 1.0

    xpk = np.zeros((128, 2, PAD), np.float32)
    xpk[:, 0, :] = xp[0:128].reshape(128, PAD)
    xpk[0:64, 1, :] = xp[128:192].reshape(64, PAD)
    xpk[64, 1, :] = ones.reshape(PAD)

    xb2 = np.zeros((65, PAD), np.float32)
    xb2[0:64] = xp[128:192].reshape(64, PAD)
    xb2[64] = ones.reshape(PAD)

    return {
        "xpk": np.ascontiguousarray(xpk.reshape(128, 2 * PAD)).astype(FP8),
        "xb1": xp[0:128].reshape(128, PAD).astype(BF16),
        "xb2": xb2.astype(BF16),
    }


def kernel(**inputs):
    import concourse.bass_utils as bu

    x = np.asarray(inputs["x"], np.float32)
    qkv_w = np.asarray(inputs["qkv_w"], np.float32)
    qkv_b = np.asarray(inputs["qkv_b"], np.float32)
    dw_w = np.asarray(inputs["dw_w"], np.float32)
    dw_b = np.asarray(inputs["dw_b"], np.float32)
    proj_w = np.asarray(inputs["proj_w"], np.float32)
    proj_b = np.asarray(inputs["proj_b"], np.float32)
    temp = np.asarray(inputs["temperature"], np.float32).reshape(HEADS)

    if "nc" not in _COMPILED:
        _COMPILED["nc"] = _build()
    nc = _COMPILED["nc"]

    common = _prep_common(qkv_w, qkv_b, dw_w, dw_b, proj_w, proj_b, temp)
    in_maps = [{**_prep_x(x[b]), **common} for b in range(x.shape[0])]
    res = bu.run_bass_kernel_spmd(nc, in_maps, core_ids=list(range(len(in_maps))))
    out = np.stack([
        r["y"].astype(np.float32).reshape(C, HW, HW) for r in res.results])
    return out
